# revision 19
# baseline (speedup 1.0000x reference)
"""BertSelfAttention (relative_key_query position embeddings) on 8 TRN2 NeuronCores.

Full inputs in, full output out.  Sharding: data-parallel over batch (4) x
tensor-parallel over head-groups (2 groups of 6 heads) = 8 cores, SPMD (one
NEFF, per-core input slices).

Math (per batch b, head h):
  q = hs @ Wq + bq ; k, v likewise            [S, 64] per head
  scores[l,r] = q[l]@k[r] + q[l]@D[l-r+M-1] + k[r]@D[l-r+M-1]
  probs = softmax(scores/8 + mask) ; ctx = probs @ v

Device algorithm (transposed orientation S[r,l], softmax over partitions):
  * The relative-position terms are handled with "band tables":
      Atab_b[p, c] = q[128b+p] . Drev_pad[896-128b+c]   (c in [0,1152))
    A row-pitch-1151 (instead of 1152) strided read of Atab yields
      qpos_b[p, r] = q[l] . D[l-r+1023]  exactly (regular DMA, on-chip).
    Same construction with D (unreversed) and k gives kposT directly in
    [r, l] orientation; qpos tiles are transpose-accumulated into the score
    PSUM via identity matmuls on the tensor engine.
  * exp((S)*0.125 + mask) fused on ACT (mask enters as per-partition bias).
  * No row-max subtraction: |scores/8| <~ 2 for this distribution, exp is
    safely in fp32 range; softmax is algebraically identical.
  * PV uses lhsT = [v | 1]: row 64 of the output accumulates the softmax
    denominator for free; division happens after the final transpose.
  * bv is folded in on the host (ctx = ctx_nobv + bv since rows of probs
    sum to 1); bq/bk are applied on-device as per-partition biases.
"""

import numpy as np
import ml_dtypes

import concourse.bass as bass
import concourse.mybir as mybir
import concourse.tile as tile
from concourse import bacc
from concourse.bass_utils import run_bass_kernel_spmd
from concourse.masks import make_identity

F32 = mybir.dt.float32
BF16 = mybir.dt.bfloat16
AF = mybir.ActivationFunctionType

B, S, H = 4, 1024, 768
NH, HD = 12, 64
MAXP = 1024
NCORES = 8
HPC = 6           # heads per core
DW = HPC * HD     # 384 out-dims per core
P = 128
NB = S // P       # 8 blocks of 128 along l and r
BAND = 1152       # band width per block (1151 needed, padded to 1152)
JW = 2048         # padded dist table width

_CACHE: dict = {}


def _diag_ap(ap: bass.AP, pitch: int, part_n: int, free_n: int, off: int) -> bass.AP:
    """View of a row-major [part_n, pitch] region reading X[p, f - p + off]."""
    d = ap.copy()
    v = d.ap
    while len(v) > 0:
        v.pop()
    v.append([pitch - 1, part_n])
    v.append([1, free_n])
    d.offset = ap.offset + off
    return d


def _diag_ap3(ap: bass.AP, nblk: int, pitch: int, part_n: int, free_n: int,
              off: int) -> bass.AP:
    """Batched diagonal view of a [part_n, nblk, pitch] tile:
    out[p, b, f] = X[p, b, f - p + off]."""
    d = ap.copy()
    v = d.ap
    while len(v) > 0:
        v.pop()
    v.append([nblk * pitch - 1, part_n])
    v.append([pitch, nblk])
    v.append([1, free_n])
    d.offset = ap.offset + off
    return d


def _build(reps: int = 1):
    key = ("nc", reps)
    if key in _CACHE:
        return _CACHE[key]

    nc = bacc.Bacc("TRN2", target_bir_lowering=False, debug=False)

    hst_d = nc.dram_tensor("hst", [P, 6, S], BF16, kind="ExternalInput")
    wq_d = nc.dram_tensor("wq", [P, 6, DW], BF16, kind="ExternalInput")
    wk_d = nc.dram_tensor("wk", [P, 6, DW], BF16, kind="ExternalInput")
    wv_d = nc.dram_tensor("wv", [P, 6, DW], BF16, kind="ExternalInput")
    drev_d = nc.dram_tensor("drevt", [P, JW], BF16, kind="ExternalInput")
    dt_d = nc.dram_tensor("dtt", [P, JW], BF16, kind="ExternalInput")
    mask_d = nc.dram_tensor("maskc", [P, NB], F32, kind="ExternalInput")
    bqk_d = nc.dram_tensor("bqkc", [P, HPC], F32, kind="ExternalInput")
    out_d = nc.dram_tensor("out", [P, NB, DW], F32, kind="ExternalOutput")

    with tile.TileContext(nc) as tc:
        with tc.tile_pool(name="persist", bufs=1) as pp:
            drevt = pp.tile([P, JW], BF16)
            dtt = pp.tile([P, JW], BF16)
            maskt = pp.tile([P, NB], F32)
            bqkt = pp.tile([P, HPC], F32)
            identb = pp.tile([P, P], BF16)
            identf = pp.tile([P, P], F32)
            # rows 0-63: q dims of head h (col h); rows 64-127: k dims
            qkT = pp.tile([P, HPC, S], BF16)
            kLo = pp.tile([64, HPC, S], BF16)   # k copy at rows 0-63 for QK
            vsb = pp.tile([P, NB, HPC, HD + 1], BF16)  # [r%128, r//128, h, d|1]
            ctxT = pp.tile([HD + 1, HPC, S], F32)
            outsb = pp.tile([P, NB, DW], F32)

            nc.sync.dma_start(drevt[:], drev_d[:])
            nc.sync.dma_start(dtt[:], dt_d[:])
            nc.sync.dma_start(maskt[:], mask_d[:])
            nc.sync.dma_start(bqkt[:], bqk_d[:])
            make_identity(nc, identb[:])
            make_identity(nc, identf[:])
            nc.vector.memset(vsb[:, :, :, HD], 1.0)

            for rep in range(reps):
                _emit_iteration(nc, tc, rep,
                                hst_d, wq_d, wk_d, wv_d, out_d,
                                drevt, dtt, maskt, bqkt,
                                identb, identf, qkT, kLo, vsb, ctxT, outsb)

    nc.compile()
    _CACHE[key] = nc
    return nc


def _emit_iteration(nc, tc, rep, hst_d, wq_d, wk_d, wv_d, out_d,
                    drevt, dtt, maskt, bqkt,
                    identb, identf, qkT, kLo, vsb, ctxT, outsb):
    # ---------------- Phase 1: projections ----------------
    with (
        tc.tile_pool(name=f"proj{rep}", bufs=1) as prp,
        tc.tile_pool(name=f"projps{rep}", bufs=2, space="PSUM") as prps,
    ):
        hst = prp.tile([P, 6, S], BF16)
        wqt = prp.tile([P, 6, DW], BF16)
        wkt = prp.tile([P, 6, DW], BF16)
        wvt = prp.tile([P, 6, DW], BF16)
        nc.sync.dma_start(hst[:], hst_d[:])
        nc.sync.dma_start(wqt[:], wq_d[:])
        nc.sync.dma_start(wkt[:], wk_d[:])
        nc.sync.dma_start(wvt[:], wv_d[:])

        # q into psum rows 0-63, k into rows 64-127 (column-packed, the two
        # matmuls of a pair run in disjoint 64-col groups concurrently)
        for h in range(HPC):
            hsl = slice(h * HD, (h + 1) * HD)
            for nh2 in range(2):
                ps = prps.tile([P, 512], F32, tag="pqk")
                for kc in range(6):
                    nc.tensor.matmul(
                        ps[0:64, :],
                        wqt[:, kc, hsl],
                        hst[:, kc, nh2 * 512:(nh2 + 1) * 512],
                        start=(kc == 0), stop=(kc == 5),
                        tile_position=(0, 0),
                        skip_group_check=True,
                    )
                    nc.tensor.matmul(
                        ps[64:P, :],
                        wkt[:, kc, hsl],
                        hst[:, kc, nh2 * 512:(nh2 + 1) * 512],
                        start=(kc == 0), stop=(kc == 5),
                        tile_position=(0, 64),
                        skip_group_check=True,
                    )
                nc.scalar.activation(
                    qkT[:, h, nh2 * 512:(nh2 + 1) * 512], ps[:],
                    AF.Identity, bias=bqkt[:, h:h + 1], scale=1.0,
                )
        # k copy to rows 0-63 for the QK matmuls (one on-chip DMA)
        nc.sync.dma_start(kLo[:], qkT[64:P, :, :])

        # v: out[M=128 tokens, N=384 outdims]
        for t in range(NB):
            ps = prps.tile([P, DW], F32, tag="pv")
            for kc in range(6):
                nc.tensor.matmul(
                    ps[:],
                    hst[:, kc, t * P:(t + 1) * P],
                    wvt[:, kc, :],
                    start=(kc == 0), stop=(kc == 5),
                )
            nc.vector.tensor_copy(
                vsb[:, t, :, 0:HD],
                ps[:].rearrange("p (h d) -> p h d", h=HPC),
            )

    # ---------------- Phase 2: per-head attention ----------------
    with (
        tc.tile_pool(name=f"wtab{rep}", bufs=1) as wtab,
        tc.tile_pool(name=f"wqpos{rep}", bufs=2) as wqpos,
        tc.tile_pool(name=f"wkpt{rep}", bufs=1) as wkpt,
        tc.tile_pool(name=f"wexp{rep}", bufs=10) as wexp,
        tc.tile_pool(name=f"hps{rep}", bufs=2, space="PSUM") as hps,
        tc.tile_pool(name=f"cps{rep}", bufs=1, space="PSUM") as cps,
    ):
        for h in range(HPC):
            qTh = qkT[0:64, h, :]
            kTh = qkT[64:P, h, :]
            kTh_lo = kLo[:, h, :]

            # --- band tables + batched on-chip diagonal shifts
            # A-side (q) on array rows 0-63, B-side (k) on rows 64-127:
            # the paired matmuls run concurrently in disjoint row groups.
            atab = wtab.tile([P, NB, BAND], BF16, tag="atab")
            btab = wtab.tile([P, NB, BAND], BF16, tag="btab")
            for blk in range(NB):
                j0 = 896 - P * blk
                bsl = slice(blk * P, (blk + 1) * P)
                ps_a = hps.tile([P, BAND], F32, tag="work")
                ps_b = hps.tile([P, BAND], F32, tag="work")
                for c0, c1 in ((0, 512), (512, 1024), (1024, BAND)):
                    nc.tensor.matmul(
                        ps_a[:, c0:c1], qTh[:, bsl],
                        drevt[0:64, j0 + c0:j0 + c1],
                        start=True, stop=True, skip_group_check=True,
                    )
                    nc.tensor.matmul(
                        ps_b[:, c0:c1], kTh[:, bsl],
                        dtt[64:P, j0 + c0:j0 + c1],
                        start=True, stop=True, skip_group_check=True,
                    )
                for side, (tab, ps) in enumerate(((atab, ps_a), (btab, ps_b))):
                    tabs = tab[:, blk, :]
                    if (blk * 2 + side) % 8 < 3:
                        nc.scalar.copy(tabs, ps[:])
                    else:
                        nc.vector.tensor_copy(tabs, ps[:])
            # one shift DMA per side per head
            qpos = wqpos.tile([P, NB, S], BF16, tag="qpos")
            kpt = wkpt.tile([P, NB, S], BF16, tag="kpt")
            nc.sync.dma_start(qpos[:], _diag_ap3(atab[:], NB, BAND, P, S, P - 1))
            nc.sync.dma_start(kpt[:], _diag_ap3(btab[:], NB, BAND, P, S, P - 1))

            # --- scores, softmax numerator
            expts = []
            for j in range(NB):
                sps = hps.tile([P, BAND], F32, tag="work")
                sS = sps[:, 0:S]
                for nh2 in range(2):
                    nc.tensor.matmul(
                        sS[:, nh2 * 512:(nh2 + 1) * 512],
                        kTh_lo[:, j * P:(j + 1) * P],
                        qTh[:, nh2 * 512:(nh2 + 1) * 512],
                        start=True, stop=False,
                        skip_group_check=True,
                    )
                for blk in range(NB):
                    nc.tensor.matmul(
                        sS[:, blk * P:(blk + 1) * P],
                        qpos[:, blk, j * P:(j + 1) * P],
                        identb[:],
                        start=False, stop=True,
                        skip_group_check=True,
                    )
                nc.vector.tensor_tensor(
                    sS, sS, kpt[:, j, :], mybir.AluOpType.add)
                expt = wexp.tile([P, S], BF16, tag="expt")
                nc.scalar.activation(
                    expt[:], sS, AF.Exp,
                    bias=maskt[:, j:j + 1], scale=0.125)
                expts.append(expt)

            # --- PV (+denominator via ones column)
            cac = cps.tile([HD + 1, S], F32, tag="ctxacc")
            for j in range(NB):
                for nh2 in range(2):
                    nc.tensor.matmul(
                        cac[:, nh2 * 512:(nh2 + 1) * 512],
                        vsb[:, j, h, :],
                        expts[j][:, nh2 * 512:(nh2 + 1) * 512],
                        start=(j == 0), stop=(j == NB - 1),
                        skip_group_check=True,
                    )
            nc.scalar.copy(ctxT[:, h, :], cac[:])

    # ---------------- Phase 3: output assembly ----------------
    with (
        tc.tile_pool(name=f"fin{rep}", bufs=4) as fin,
        tc.tile_pool(name=f"fps{rep}", bufs=4, space="PSUM") as fps,
    ):
        for h in range(HPC):
            for lt in range(NB):
                ct = fps.tile([P, HD + 1], F32, tag="ctps")
                nc.tensor.matmul(
                    ct[:],
                    ctxT[:, h, lt * P:(lt + 1) * P],
                    identf[0:HD + 1, 0:HD + 1],
                    start=True, stop=True,
                    skip_group_check=True,
                )
                rc = fin.tile([P, 1], F32, tag="rc")
                nc.vector.reciprocal(rc[:], ct[:, HD:HD + 1])
                nc.vector.tensor_scalar_mul(
                    outsb[:, lt, h * HD:(h + 1) * HD],
                    ct[:, 0:HD], rc[:])
        nc.sync.dma_start(out_d[:], outsb[:])


def build_in_maps(inputs):
    hs = np.asarray(inputs["hidden_states"], np.float32)
    am = np.asarray(inputs["attention_mask"], np.float32)
    Wq = np.asarray(inputs["Wq"], np.float32)
    Wk = np.asarray(inputs["Wk"], np.float32)
    Wv = np.asarray(inputs["Wv"], np.float32)
    bq = np.asarray(inputs["bq"], np.float32)
    bk = np.asarray(inputs["bk"], np.float32)
    de = np.asarray(inputs["dist_emb"], np.float32)

    bf = ml_dtypes.bfloat16

    # dist tables, padded to 2048 cols, duplicated on both partition halves
    drevt = np.zeros((64, JW), np.float32)
    drevt[:, :2047] = de[::-1].T
    dtt = np.zeros((64, JW), np.float32)
    dtt[:, :2047] = de.T
    drevt = np.concatenate([drevt, drevt], 0).astype(bf)
    dtt = np.concatenate([dtt, dtt], 0).astype(bf)

    in_maps = []
    for core in range(NCORES):
        b, g = divmod(core, 2)
        cols = slice(g * DW, (g + 1) * DW)
        hst = np.ascontiguousarray(hs[b].T).reshape(6, P, S)
        hst = np.ascontiguousarray(hst.transpose(1, 0, 2)).astype(bf)
        wqc = np.ascontiguousarray(
            Wq[:, cols].reshape(6, P, DW).transpose(1, 0, 2)).astype(bf)
        wkc = np.ascontiguousarray(
            Wk[:, cols].reshape(6, P, DW).transpose(1, 0, 2)).astype(bf)
        wvc = np.ascontiguousarray(
            Wv[:, cols].reshape(6, P, DW).transpose(1, 0, 2)).astype(bf)
        maskc = np.ascontiguousarray(am[b, 0, 0, :].reshape(NB, P).T)
        # rows 0-63: bq dims of head h (col h); rows 64-127: bk dims
        bqkc = np.concatenate(
            [bq[cols].reshape(HPC, HD).T, bk[cols].reshape(HPC, HD).T], axis=0)
        in_maps.append({
            "hst": hst, "wq": wqc, "wk": wkc, "wv": wvc,
            "drevt": drevt, "dtt": dtt,
            "maskc": maskc.astype(np.float32),
            "bqkc": np.ascontiguousarray(bqkc).astype(np.float32),
        })
    return in_maps


def kernel(hidden_states, attention_mask, Wq, bq, Wk, bk, Wv, bv, dist_emb):
    in_maps = build_in_maps({
        "hidden_states": hidden_states, "attention_mask": attention_mask,
        "Wq": Wq, "Wk": Wk, "Wv": Wv, "bq": bq, "bk": bk,
        "dist_emb": dist_emb,
    })
    bv = np.asarray(bv, np.float32)

    nc = _build()
    res = run_bass_kernel_spmd(nc, in_maps, core_ids=list(range(NCORES)))

    out = np.empty((B, S, H), np.float32)
    for core in range(NCORES):
        b, g = divmod(core, 2)
        o = res.results[core]["out"]          # [128, 8, 384]
        out[b, :, g * DW:(g + 1) * DW] = o.transpose(1, 0, 2).reshape(S, DW)
    out += bv[None, None, :]
    return out


# revision 21
# speedup vs baseline: 2.6495x; 2.6495x over previous
"""BertSelfAttention (relative_key_query position embeddings) on 8 TRN2 NeuronCores.

Full inputs in, full output out.  Sharding: data-parallel over batch (4) x
tensor-parallel over head-groups (2 groups of 6 heads) = 8 cores, SPMD (one
NEFF, per-core input slices).

Math (per batch b, head h):
  q = hs @ Wq + bq ; k, v likewise            [S, 64] per head
  scores[l,r] = q[l]@k[r] + q[l]@D[l-r+M-1] + k[r]@D[l-r+M-1]
  probs = softmax(scores/8 + mask) ; ctx = probs @ v

Device algorithm (transposed orientation S[r,l], softmax over partitions):
  * The relative-position terms are handled with "band tables":
      Atab_b[p, c] = q[128b+p] . Drev_pad[896-128b+c]   (c in [0,1152))
    A row-pitch-1151 (instead of 1152) strided read of Atab yields
      qpos_b[p, r] = q[l] . D[l-r+1023]  exactly (regular DMA, on-chip,
    batched: one SBUF->SBUF DMA per table per head).
    Same construction with D (unreversed) and k gives kposT directly in
    [r, l] orientation; qpos tiles are transpose-accumulated into the score
    PSUM via identity matmuls on the tensor engine.
  * exp((S)*0.125 + mask) fused on ACT (mask enters as per-partition bias).
  * No row-max subtraction: |scores/8| <~ 2 for this distribution, exp is
    safely in fp32 range; softmax is algebraically identical.
  * PV uses lhsT = [v | 1]: row 64 of the output accumulates the softmax
    denominator for free; division happens after the final transpose.
  * bv is folded in on the host (ctx = ctx_nobv + bv since rows of probs
    sum to 1); bq/bk are applied on-device as per-partition biases.
"""

import numpy as np
import ml_dtypes

import concourse.bass as bass
import concourse.mybir as mybir
import concourse.tile as tile
from concourse import bacc
from concourse.bass_utils import run_bass_kernel_spmd
from concourse.masks import make_identity

F32 = mybir.dt.float32
BF16 = mybir.dt.bfloat16
AF = mybir.ActivationFunctionType

B, S, H = 4, 1024, 768
NH, HD = 12, 64
MAXP = 1024
NCORES = 8
HPC = 6           # heads per core
DW = HPC * HD     # 384 out-dims per core
P = 128
NB = S // P       # 8 blocks of 128 along l and r
BAND = 1152       # band width per block (1151 needed, padded to 1152)
JW = 2048         # padded dist table width

_CACHE: dict = {}


def _diag_ap3(ap: bass.AP, nblk: int, pitch: int, part_n: int, free_n: int,
              off: int) -> bass.AP:
    """Batched diagonal view of a [part_n, nblk, pitch] tile:
    out[p, b, f] = X[p, b, f - p + off]."""
    d = ap.copy()
    v = d.ap
    while len(v) > 0:
        v.pop()
    v.append([nblk * pitch - 1, part_n])
    v.append([pitch, nblk])
    v.append([1, free_n])
    d.offset = ap.offset + off
    return d


def _build(reps: int = 1):
    key = ("nc", reps)
    if key in _CACHE:
        return _CACHE[key]

    nc = bacc.Bacc("TRN2", target_bir_lowering=False, debug=False)

    hst_d = nc.dram_tensor("hst", [P, 6, S], BF16, kind="ExternalInput")
    wq_d = nc.dram_tensor("wq", [P, 6, DW], BF16, kind="ExternalInput")
    wk_d = nc.dram_tensor("wk", [P, 6, DW], BF16, kind="ExternalInput")
    wv_d = nc.dram_tensor("wv", [P, 6, DW], BF16, kind="ExternalInput")
    drev_d = nc.dram_tensor("drevt", [P, JW], BF16, kind="ExternalInput")
    dt_d = nc.dram_tensor("dtt", [P, JW], BF16, kind="ExternalInput")
    mask_d = nc.dram_tensor("maskc", [P, NB], F32, kind="ExternalInput")
    bqk_d = nc.dram_tensor("bqkc", [P, 6], F32, kind="ExternalInput")
    out_d = nc.dram_tensor("out", [P, NB, DW], F32, kind="ExternalOutput")

    with tile.TileContext(nc) as tc:
        with tc.tile_pool(name="persist", bufs=1) as pp:
            drevt = pp.tile([P, JW], BF16)
            dtt = pp.tile([P, JW], BF16)
            maskt = pp.tile([P, NB], F32)
            bqkt = pp.tile([P, 6], F32)
            identb = pp.tile([P, P], BF16)
            identf = pp.tile([P, P], F32)
            qT = pp.tile([P, 3, S], BF16)     # head h: [64*(h%2):.., h//2, :]
            kT = pp.tile([P, 3, S], BF16)
            vsb = pp.tile([P, NB, HPC, HD + 1], BF16)  # [r%128, r//128, h, d|1]
            ctxT = pp.tile([HD + 1, HPC, S], F32)
            outsb = pp.tile([P, NB, DW], F32)

            nc.sync.dma_start(drevt[:], drev_d[:])
            nc.sync.dma_start(dtt[:], dt_d[:])
            nc.sync.dma_start(maskt[:], mask_d[:])
            nc.sync.dma_start(bqkt[:], bqk_d[:])
            make_identity(nc, identb[:])
            make_identity(nc, identf[:])
            nc.vector.memset(vsb[:, :, :, HD], 1.0)

            for rep in range(reps):
                _emit_iteration(nc, tc, rep,
                                hst_d, wq_d, wk_d, wv_d, out_d,
                                drevt, dtt, maskt, bqkt,
                                identb, identf, qT, kT, vsb, ctxT, outsb)

    nc.compile()
    _CACHE[key] = nc
    return nc


def _emit_iteration(nc, tc, rep, hst_d, wq_d, wk_d, wv_d, out_d,
                    drevt, dtt, maskt, bqkt,
                    identb, identf, qT, kT, vsb, ctxT, outsb):
    # ---------------- Phase 1: projections ----------------
    with (
        tc.tile_pool(name=f"proj{rep}", bufs=1) as prp,
        tc.tile_pool(name=f"projps{rep}", bufs=2, space="PSUM") as prps,
    ):
        hst = prp.tile([P, 6, S], BF16)
        wqt = prp.tile([P, 6, DW], BF16)
        wkt = prp.tile([P, 6, DW], BF16)
        wvt = prp.tile([P, 6, DW], BF16)
        nc.sync.dma_start(hst[:], hst_d[:])
        nc.sync.dma_start(wqt[:], wq_d[:])
        nc.sync.dma_start(wkt[:], wk_d[:])
        nc.sync.dma_start(wvt[:], wv_d[:])

        # qT / kT: out[M=128 outdims (2 heads), N=512 tokens]
        for pi, (wt, dst) in enumerate(((wqt, qT), (wkt, kT))):
            for m in range(3):
                for nh2 in range(2):
                    ps = prps.tile([P, 512], F32, tag="pqk")
                    for kc in range(6):
                        nc.tensor.matmul(
                            ps[:],
                            wt[:, kc, m * P:(m + 1) * P],
                            hst[:, kc, nh2 * 512:(nh2 + 1) * 512],
                            start=(kc == 0), stop=(kc == 5),
                        )
                    nc.scalar.activation(
                        dst[:, m, nh2 * 512:(nh2 + 1) * 512], ps[:],
                        AF.Identity, bias=bqkt[:, 3 * pi + m:3 * pi + m + 1],
                        scale=1.0,
                    )
        # v: out[M=128 tokens, N=384 outdims]
        for t in range(NB):
            ps = prps.tile([P, DW], F32, tag="pv")
            for kc in range(6):
                nc.tensor.matmul(
                    ps[:],
                    hst[:, kc, t * P:(t + 1) * P],
                    wvt[:, kc, :],
                    start=(kc == 0), stop=(kc == 5),
                )
            nc.vector.tensor_copy(
                vsb[:, t, :, 0:HD],
                ps[:].rearrange("p (h d) -> p h d", h=HPC),
            )

    # ---------------- Phase 2: per-head attention ----------------
    with (
        tc.tile_pool(name=f"wtab{rep}", bufs=1) as wtab,
        tc.tile_pool(name=f"wqpos{rep}", bufs=2) as wqpos,
        tc.tile_pool(name=f"wkpt{rep}", bufs=1) as wkpt,
        tc.tile_pool(name=f"wexp{rep}", bufs=10) as wexp,
        tc.tile_pool(name=f"hps{rep}", bufs=2, space="PSUM") as hps,
        tc.tile_pool(name=f"cps{rep}", bufs=1, space="PSUM") as cps,
    ):
        for h in range(HPC):
            base = (h % 2) * 64
            tl = h // 2
            qTh = qT[base:base + 64, tl, :]
            kTh = kT[base:base + 64, tl, :]

            # --- band tables + batched on-chip diagonal shifts
            atab = wtab.tile([P, NB, BAND], BF16, tag="atab")
            btab = wtab.tile([P, NB, BAND], BF16, tag="btab")
            for blk in range(NB):
                j0 = 896 - P * blk
                for side in range(2):
                    ps = hps.tile([P, BAND], F32, tag="work")
                    lhsT = (qTh if side == 0 else kTh)[:, blk * P:(blk + 1) * P]
                    rhs_t = (drevt if side == 0 else dtt)[base:base + 64, :]
                    for c0, c1 in ((0, 512), (512, 1024), (1024, BAND)):
                        nc.tensor.matmul(
                            ps[:, c0:c1], lhsT,
                            rhs_t[:, j0 + c0:j0 + c1],
                            start=True, stop=True,
                            skip_group_check=True,
                        )
                    tabs = (atab if side == 0 else btab)[:, blk, :]
                    if (blk * 2 + side) % 8 < 3:
                        nc.scalar.copy(tabs, ps[:])
                    else:
                        nc.vector.tensor_copy(tabs, ps[:])
            # one shift DMA per side per head
            qpos = wqpos.tile([P, NB, S], BF16, tag="qpos")
            kpt = wkpt.tile([P, NB, S], BF16, tag="kpt")
            nc.sync.dma_start(qpos[:], _diag_ap3(atab[:], NB, BAND, P, S, P - 1))
            nc.sync.dma_start(kpt[:], _diag_ap3(btab[:], NB, BAND, P, S, P - 1))

            # --- scores, softmax numerator
            expts = []
            for j in range(NB):
                sps = hps.tile([P, BAND], F32, tag="work")
                sS = sps[:, 0:S]
                for nh2 in range(2):
                    nc.tensor.matmul(
                        sS[:, nh2 * 512:(nh2 + 1) * 512],
                        kTh[:, j * P:(j + 1) * P],
                        qTh[:, nh2 * 512:(nh2 + 1) * 512],
                        start=True, stop=False,
                        skip_group_check=True,
                    )
                for blk in range(NB):
                    nc.tensor.matmul(
                        sS[:, blk * P:(blk + 1) * P],
                        qpos[:, blk, j * P:(j + 1) * P],
                        identb[:],
                        start=False, stop=True,
                        skip_group_check=True,
                    )
                nc.vector.tensor_tensor(
                    sS, sS, kpt[:, j, :], mybir.AluOpType.add)
                expt = wexp.tile([P, S], BF16, tag="expt")
                nc.scalar.activation(
                    expt[:], sS, AF.Exp,
                    bias=maskt[:, j:j + 1], scale=0.125)
                expts.append(expt)

            # --- PV (+denominator via ones column)
            cac = cps.tile([HD + 1, S], F32, tag="ctxacc")
            for j in range(NB):
                for nh2 in range(2):
                    nc.tensor.matmul(
                        cac[:, nh2 * 512:(nh2 + 1) * 512],
                        vsb[:, j, h, :],
                        expts[j][:, nh2 * 512:(nh2 + 1) * 512],
                        start=(j == 0), stop=(j == NB - 1),
                        skip_group_check=True,
                    )
            nc.scalar.copy(ctxT[:, h, :], cac[:])

    # ---------------- Phase 3: output assembly ----------------
    with (
        tc.tile_pool(name=f"fin{rep}", bufs=4) as fin,
        tc.tile_pool(name=f"fps{rep}", bufs=4, space="PSUM") as fps,
    ):
        for h in range(HPC):
            for lt in range(NB):
                ct = fps.tile([P, HD + 1], F32, tag="ctps")
                nc.tensor.matmul(
                    ct[:],
                    ctxT[:, h, lt * P:(lt + 1) * P],
                    identf[0:HD + 1, 0:HD + 1],
                    start=True, stop=True,
                    skip_group_check=True,
                )
                rc = fin.tile([P, 1], F32, tag="rc")
                nc.vector.reciprocal(rc[:], ct[:, HD:HD + 1])
                nc.vector.tensor_scalar_mul(
                    outsb[:, lt, h * HD:(h + 1) * HD],
                    ct[:, 0:HD], rc[:])
        nc.sync.dma_start(out_d[:], outsb[:])


def build_in_maps(inputs):
    hs = np.asarray(inputs["hidden_states"], np.float32)
    am = np.asarray(inputs["attention_mask"], np.float32)
    Wq = np.asarray(inputs["Wq"], np.float32)
    Wk = np.asarray(inputs["Wk"], np.float32)
    Wv = np.asarray(inputs["Wv"], np.float32)
    bq = np.asarray(inputs["bq"], np.float32)
    bk = np.asarray(inputs["bk"], np.float32)
    de = np.asarray(inputs["dist_emb"], np.float32)

    bf = ml_dtypes.bfloat16

    # dist tables, padded to 2048 cols, duplicated on both partition halves
    drevt = np.zeros((64, JW), np.float32)
    drevt[:, :2047] = de[::-1].T
    dtt = np.zeros((64, JW), np.float32)
    dtt[:, :2047] = de.T
    drevt = np.concatenate([drevt, drevt], 0).astype(bf)
    dtt = np.concatenate([dtt, dtt], 0).astype(bf)

    in_maps = []
    for core in range(NCORES):
        b, g = divmod(core, 2)
        cols = slice(g * DW, (g + 1) * DW)
        hst = np.ascontiguousarray(hs[b].T).reshape(6, P, S)
        hst = np.ascontiguousarray(hst.transpose(1, 0, 2)).astype(bf)
        wqc = np.ascontiguousarray(
            Wq[:, cols].reshape(6, P, DW).transpose(1, 0, 2)).astype(bf)
        wkc = np.ascontiguousarray(
            Wk[:, cols].reshape(6, P, DW).transpose(1, 0, 2)).astype(bf)
        wvc = np.ascontiguousarray(
            Wv[:, cols].reshape(6, P, DW).transpose(1, 0, 2)).astype(bf)
        maskc = np.ascontiguousarray(am[b, 0, 0, :].reshape(NB, P).T)
        # col 0..2: bq m-tiles (128 dims each); col 3..5: bk m-tiles
        bqkc = np.concatenate(
            [bq[cols].reshape(3, P).T, bk[cols].reshape(3, P).T], axis=1)
        in_maps.append({
            "hst": hst, "wq": wqc, "wk": wkc, "wv": wvc,
            "drevt": drevt, "dtt": dtt,
            "maskc": maskc.astype(np.float32),
            "bqkc": np.ascontiguousarray(bqkc).astype(np.float32),
        })
    return in_maps


def kernel(hidden_states, attention_mask, Wq, bq, Wk, bk, Wv, bv, dist_emb):
    in_maps = build_in_maps({
        "hidden_states": hidden_states, "attention_mask": attention_mask,
        "Wq": Wq, "Wk": Wk, "Wv": Wv, "bq": bq, "bk": bk,
        "dist_emb": dist_emb,
    })
    bv = np.asarray(bv, np.float32)

    nc = _build()
    res = run_bass_kernel_spmd(nc, in_maps, core_ids=list(range(NCORES)))

    out = np.empty((B, S, H), np.float32)
    for core in range(NCORES):
        b, g = divmod(core, 2)
        o = res.results[core]["out"]          # [128, 8, 384]
        out[b, :, g * DW:(g + 1) * DW] = o.transpose(1, 0, 2).reshape(S, DW)
    out += bv[None, None, :]
    return out


# revision 22
# speedup vs baseline: 4.9091x; 1.8528x over previous
"""BertSelfAttention (relative_key_query position embeddings) on 8 TRN2 NeuronCores.

Full inputs in, full output out.  Sharding: data-parallel over batch (4) x
tensor-parallel over head-groups (2 groups of 6 heads) = 8 cores, SPMD (one
NEFF, per-core input slices).

Math (per batch b, head h):
  q = hs @ Wq + bq ; k, v likewise            [S, 64] per head
  scores[l,r] = q[l]@k[r] + q[l]@D[l-r+M-1] + k[r]@D[l-r+M-1]
  probs = softmax(scores/8 + mask) ; ctx = probs @ v

Device algorithm (transposed orientation S[r,l], softmax over partitions):
  * The relative-position terms are handled with "band tables":
      Atab_b[p, c] = q[128b+p] . Drev_pad[896-128b+c]   (c in [0,1152))
    A row-pitch-1151 (instead of 1152) strided read of Atab yields
      qpos_b[p, r] = q[l] . D[l-r+1023]  exactly (regular DMA, on-chip,
    batched: one SBUF->SBUF DMA per table per head).
    Same construction with D (unreversed) and k gives kposT directly in
    [r, l] orientation; qpos tiles are transpose-accumulated into the score
    PSUM via identity matmuls on the tensor engine.
  * exp((S)*0.125 + mask) fused on ACT (mask enters as per-partition bias).
  * No row-max subtraction: |scores/8| <~ 2 for this distribution, exp is
    safely in fp32 range; softmax is algebraically identical.
  * PV uses lhsT = [v | 1]: row 64 of the output accumulates the softmax
    denominator for free; division happens after the final transpose.
  * bv is folded in on the host (ctx = ctx_nobv + bv since rows of probs
    sum to 1); bq/bk are applied on-device as per-partition biases.
"""

import numpy as np
import ml_dtypes

import concourse.bass as bass
import concourse.mybir as mybir
import concourse.tile as tile
from concourse import bacc
from concourse.bass_utils import run_bass_kernel_spmd
from concourse.masks import make_identity

F32 = mybir.dt.float32
BF16 = mybir.dt.bfloat16
AF = mybir.ActivationFunctionType

B, S, H = 4, 1024, 768
NH, HD = 12, 64
MAXP = 1024
NCORES = 8
HPC = 6           # heads per core
DW = HPC * HD     # 384 out-dims per core
P = 128
NB = S // P       # 8 blocks of 128 along l and r
BAND = 1152       # band width per block (1151 needed, padded to 1152)
JW = 2048         # padded dist table width

_CACHE: dict = {}


def _diag_ap3(ap: bass.AP, nblk: int, pitch: int, part_n: int, free_n: int,
              off: int) -> bass.AP:
    """Batched diagonal view of a [part_n, nblk, pitch] tile:
    out[p, b, f] = X[p, b, f - p + off]."""
    d = ap.copy()
    v = d.ap
    while len(v) > 0:
        v.pop()
    v.append([nblk * pitch - 1, part_n])
    v.append([pitch, nblk])
    v.append([1, free_n])
    d.offset = ap.offset + off
    return d


def _build(reps: int = 1):
    key = ("nc", reps)
    if key in _CACHE:
        return _CACHE[key]

    nc = bacc.Bacc("TRN2", target_bir_lowering=False, debug=False)

    hst_d = nc.dram_tensor("hst", [P, 6, S], BF16, kind="ExternalInput")
    wq_d = nc.dram_tensor("wq", [P, 6, DW], BF16, kind="ExternalInput")
    wk_d = nc.dram_tensor("wk", [P, 6, DW], BF16, kind="ExternalInput")
    wv_d = nc.dram_tensor("wv", [P, 6, DW], BF16, kind="ExternalInput")
    drev_d = nc.dram_tensor("drevt", [P, JW], BF16, kind="ExternalInput")
    dt_d = nc.dram_tensor("dtt", [P, JW], BF16, kind="ExternalInput")
    mask_d = nc.dram_tensor("maskc", [P, NB], F32, kind="ExternalInput")
    bqk_d = nc.dram_tensor("bqkc", [P, 6], F32, kind="ExternalInput")
    out_d = nc.dram_tensor("out", [P, NB, DW], F32, kind="ExternalOutput")

    with tile.TileContext(nc) as tc:
        with tc.tile_pool(name="persist", bufs=1) as pp:
            drevt = pp.tile([P, JW], BF16)
            dtt = pp.tile([P, JW], BF16)
            maskt = pp.tile([P, NB], F32)
            bqkt = pp.tile([P, 6], F32)
            identb = pp.tile([P, P], BF16)
            identf = pp.tile([P, P], F32)
            qT = pp.tile([P, 3, S], BF16)     # head h: [64*(h%2):.., h//2, :]
            kT = pp.tile([P, 3, S], BF16)
            vsb = pp.tile([P, NB, HPC, HD + 1], BF16)  # [r%128, r//128, h, d|1]
            ctxT = pp.tile([HD + 1, HPC, S], F32)
            outsb = pp.tile([P, NB, DW], F32)

            nc.sync.dma_start(drevt[:], drev_d[:])
            nc.sync.dma_start(dtt[:], dt_d[:])
            nc.sync.dma_start(maskt[:], mask_d[:])
            nc.sync.dma_start(bqkt[:], bqk_d[:])
            make_identity(nc, identb[:])
            make_identity(nc, identf[:])
            nc.vector.memset(vsb[:, :, :, HD], 1.0)

            for rep in range(reps):
                _emit_iteration(nc, tc, rep,
                                hst_d, wq_d, wk_d, wv_d, out_d,
                                drevt, dtt, maskt, bqkt,
                                identb, identf, qT, kT, vsb, ctxT, outsb)

    nc.compile()
    _CACHE[key] = nc
    return nc


def _emit_iteration(nc, tc, rep, hst_d, wq_d, wk_d, wv_d, out_d,
                    drevt, dtt, maskt, bqkt,
                    identb, identf, qT, kT, vsb, ctxT, outsb):
    # ---------------- Phase 1: projections ----------------
    with (
        tc.tile_pool(name=f"proj{rep}", bufs=1) as prp,
        tc.tile_pool(name=f"projps{rep}", bufs=2, space="PSUM") as prps,
    ):
        hst = prp.tile([P, 6, S], BF16)
        wqt = prp.tile([P, 6, DW], BF16)
        wkt = prp.tile([P, 6, DW], BF16)
        wvt = prp.tile([P, 6, DW], BF16)
        nc.sync.dma_start(hst[:], hst_d[:])
        nc.sync.dma_start(wqt[:], wq_d[:])
        nc.sync.dma_start(wkt[:], wk_d[:])
        nc.sync.dma_start(wvt[:], wv_d[:])

        # qT / kT: out[M=128 outdims (2 heads), N=512 tokens]
        for pi, (wt, dst) in enumerate(((wqt, qT), (wkt, kT))):
            for m in range(3):
                for nh2 in range(2):
                    ps = prps.tile([P, 512], F32, tag="pqk")
                    for kc in range(6):
                        nc.tensor.matmul(
                            ps[:],
                            wt[:, kc, m * P:(m + 1) * P],
                            hst[:, kc, nh2 * 512:(nh2 + 1) * 512],
                            start=(kc == 0), stop=(kc == 5),
                        )
                    nc.scalar.activation(
                        dst[:, m, nh2 * 512:(nh2 + 1) * 512], ps[:],
                        AF.Identity, bias=bqkt[:, 3 * pi + m:3 * pi + m + 1],
                        scale=1.0,
                    )
        # v: out[M=128 tokens, N=384 outdims]
        for t in range(NB):
            ps = prps.tile([P, DW], F32, tag="pv")
            for kc in range(6):
                nc.tensor.matmul(
                    ps[:],
                    hst[:, kc, t * P:(t + 1) * P],
                    wvt[:, kc, :],
                    start=(kc == 0), stop=(kc == 5),
                )
            nc.vector.tensor_copy(
                vsb[:, t, :, 0:HD],
                ps[:].rearrange("p (h d) -> p h d", h=HPC),
            )

    # ---------------- Phase 2: per-head attention ----------------
    with (
        tc.tile_pool(name=f"wtab{rep}", bufs=1) as wtab,
        tc.tile_pool(name=f"wqpos{rep}", bufs=2) as wqpos,
        tc.tile_pool(name=f"wkpt{rep}", bufs=1) as wkpt,
        tc.tile_pool(name=f"wexp{rep}", bufs=10) as wexp,
        tc.tile_pool(name=f"hps{rep}", bufs=2, space="PSUM") as hps,
        tc.tile_pool(name=f"cps{rep}", bufs=1, space="PSUM") as cps,
    ):
        for h in range(HPC):
            base = (h % 2) * 64
            tl = h // 2
            qTh = qT[base:base + 64, tl, :]
            kTh = kT[base:base + 64, tl, :]

            # --- band tables + batched on-chip diagonal shifts
            atab = wtab.tile([P, NB, BAND], BF16, tag="atab")
            btab = wtab.tile([P, NB, BAND], BF16, tag="btab")
            for blk in range(NB):
                j0 = 896 - P * blk
                for side in range(2):
                    ps = hps.tile([P, BAND], F32, tag="work")
                    lhsT = (qTh if side == 0 else kTh)[:, blk * P:(blk + 1) * P]
                    rhs_t = (drevt if side == 0 else dtt)[base:base + 64, :]
                    for c0, c1 in ((0, 512), (512, 1024), (1024, BAND)):
                        nc.tensor.matmul(
                            ps[:, c0:c1], lhsT,
                            rhs_t[:, j0 + c0:j0 + c1],
                            start=True, stop=True,
                            skip_group_check=True,
                        )
                    tabs = (atab if side == 0 else btab)[:, blk, :]
                    if (blk * 2 + side) % 8 < 3:
                        nc.scalar.copy(tabs, ps[:])
                    else:
                        nc.vector.tensor_copy(tabs, ps[:])
            # one shift DMA per side per head
            qpos = wqpos.tile([P, NB, S], BF16, tag="qpos")
            kpt = wkpt.tile([P, NB, S], BF16, tag="kpt")
            nc.sync.dma_start(qpos[:], _diag_ap3(atab[:], NB, BAND, P, S, P - 1))
            nc.sync.dma_start(kpt[:], _diag_ap3(btab[:], NB, BAND, P, S, P - 1))

            # --- scores, softmax numerator
            expts = []
            for j in range(NB):
                sps = hps.tile([P, BAND], F32, tag="work")
                sS = sps[:, 0:S]
                for nh2 in range(2):
                    nc.tensor.matmul(
                        sS[:, nh2 * 512:(nh2 + 1) * 512],
                        kTh[:, j * P:(j + 1) * P],
                        qTh[:, nh2 * 512:(nh2 + 1) * 512],
                        start=True, stop=False,
                        skip_group_check=True,
                    )
                for blk in range(NB):
                    nc.tensor.matmul(
                        sS[:, blk * P:(blk + 1) * P],
                        qpos[:, blk, j * P:(j + 1) * P],
                        identb[:],
                        start=False, stop=True,
                        skip_group_check=True,
                    )
                nc.vector.tensor_tensor(
                    sS, sS, kpt[:, j, :], mybir.AluOpType.add)
                expt = wexp.tile([P, S], BF16, tag="expt")
                nc.scalar.activation(
                    expt[:], sS, AF.Exp,
                    bias=maskt[:, j:j + 1], scale=0.125)
                expts.append(expt)

            # --- PV (+denominator via ones column)
            cac = cps.tile([HD + 1, S], F32, tag="ctxacc")
            for j in range(NB):
                for nh2 in range(2):
                    nc.tensor.matmul(
                        cac[:, nh2 * 512:(nh2 + 1) * 512],
                        vsb[:, j, h, :],
                        expts[j][:, nh2 * 512:(nh2 + 1) * 512],
                        start=(j == 0), stop=(j == NB - 1),
                        skip_group_check=True,
                    )
            nc.scalar.copy(ctxT[:, h, :], cac[:])

    # ---------------- Phase 3: output assembly ----------------
    with (
        tc.tile_pool(name=f"fin{rep}", bufs=4) as fin,
        tc.tile_pool(name=f"fps{rep}", bufs=4, space="PSUM") as fps,
    ):
        for h in range(HPC):
            for lt in range(NB):
                ct = fps.tile([P, HD + 1], F32, tag="ctps")
                nc.tensor.matmul(
                    ct[:],
                    ctxT[:, h, lt * P:(lt + 1) * P],
                    identf[0:HD + 1, 0:HD + 1],
                    start=True, stop=True,
                    skip_group_check=True,
                )
                rc = fin.tile([P, 1], F32, tag="rc")
                nc.vector.reciprocal(rc[:], ct[:, HD:HD + 1])
                nc.vector.tensor_scalar_mul(
                    outsb[:, lt, h * HD:(h + 1) * HD],
                    ct[:, 0:HD], rc[:])
        nc.sync.dma_start(out_d[:], outsb[:])


def build_in_maps(inputs):
    hs = np.asarray(inputs["hidden_states"], np.float32)
    am = np.asarray(inputs["attention_mask"], np.float32)
    Wq = np.asarray(inputs["Wq"], np.float32)
    Wk = np.asarray(inputs["Wk"], np.float32)
    Wv = np.asarray(inputs["Wv"], np.float32)
    bq = np.asarray(inputs["bq"], np.float32)
    bk = np.asarray(inputs["bk"], np.float32)
    de = np.asarray(inputs["dist_emb"], np.float32)

    bf = ml_dtypes.bfloat16

    # dist tables, padded to 2048 cols, duplicated on both partition halves
    drevt = np.zeros((64, JW), np.float32)
    drevt[:, :2047] = de[::-1].T
    dtt = np.zeros((64, JW), np.float32)
    dtt[:, :2047] = de.T
    drevt = np.concatenate([drevt, drevt], 0).astype(bf)
    dtt = np.concatenate([dtt, dtt], 0).astype(bf)

    in_maps = []
    for core in range(NCORES):
        b, g = divmod(core, 2)
        cols = slice(g * DW, (g + 1) * DW)
        hst = np.ascontiguousarray(hs[b].T).reshape(6, P, S)
        hst = np.ascontiguousarray(hst.transpose(1, 0, 2)).astype(bf)
        wqc = np.ascontiguousarray(
            Wq[:, cols].reshape(6, P, DW).transpose(1, 0, 2)).astype(bf)
        wkc = np.ascontiguousarray(
            Wk[:, cols].reshape(6, P, DW).transpose(1, 0, 2)).astype(bf)
        wvc = np.ascontiguousarray(
            Wv[:, cols].reshape(6, P, DW).transpose(1, 0, 2)).astype(bf)
        maskc = np.ascontiguousarray(am[b, 0, 0, :].reshape(NB, P).T)
        # col 0..2: bq m-tiles (128 dims each); col 3..5: bk m-tiles
        bqkc = np.concatenate(
            [bq[cols].reshape(3, P).T, bk[cols].reshape(3, P).T], axis=1)
        in_maps.append({
            "hst": hst, "wq": wqc, "wk": wkc, "wv": wvc,
            "drevt": drevt, "dtt": dtt,
            "maskc": maskc.astype(np.float32),
            "bqkc": np.ascontiguousarray(bqkc).astype(np.float32),
        })
    return in_maps


def kernel(hidden_states, attention_mask, Wq, bq, Wk, bk, Wv, bv, dist_emb):
    in_maps = build_in_maps({
        "hidden_states": hidden_states, "attention_mask": attention_mask,
        "Wq": Wq, "Wk": Wk, "Wv": Wv, "bq": bq, "bk": bk,
        "dist_emb": dist_emb,
    })
    bv = np.asarray(bv, np.float32)

    nc = _build()
    try:
        res = run_bass_kernel_spmd(nc, in_maps, core_ids=list(range(NCORES)))
    except Exception:
        # one retry for transient runtime/device hiccups
        res = run_bass_kernel_spmd(nc, in_maps, core_ids=list(range(NCORES)))

    out = np.empty((B, S, H), np.float32)
    for core in range(NCORES):
        b, g = divmod(core, 2)
        o = res.results[core]["out"]          # [128, 8, 384]
        out[b, :, g * DW:(g + 1) * DW] = o.transpose(1, 0, 2).reshape(S, DW)
    out += bv[None, None, :]
    return out
